# revision 17
# baseline (speedup 1.0000x reference)
"""DisenGCN Trainium2 kernel (8 NeuronCores, Bass/Tile), v2.

Strategy
--------
Nodes are sharded across 8 cores by contiguous ranges balanced on in-edge
count.  Host-side index prep packs each node's in-edges into fixed-width
rows (J slots) and packs nodes into 128-row chunks (<= NPC nodes per
chunk); chunks are processed in groups of B=4.  On device:

  - projection fac = l2norm(leaky_relu(emb @ (W+b))) runs per 512-node
    macro-tile; the per-factor rsqrt uses a DVE integer bit-trick plus one
    Newton step (keeps the ACT engine pinned to the `exp_and_others`
    activation-table set: Prelu/Square/Exp/Copy only -- no table reloads).
  - fac is AllGathered once in bf16; per group, ONE multi-offset indirect
    DMA fetches all B*J*128 tail rows (the ~1us SWDGE fixed cost is paid
    per group instead of per (chunk, slot)).
  - both routing iterations run back-to-back per group: tails and the
    fac window live in SBUF, iteration 0's normalized output feeds
    iteration 1's head matmul directly -- no DRAM round-trip.
  - heads are expanded from the <=NPC-node window with a one-hot PE
    matmul; per-node aggregation uses the transposed one-hot.  Both
    one-hot matrices are precomputed on the host in bf16 and streamed.
  - all big elementwise/reduce work runs in bf16 on DVE (2x packing);
    softmax exp and the squares run on ACT.

The only collective is one AllGather of the projected bf16 fac table.
"""

import os
import sys

import numpy as np

sys.path.insert(0, "/opt/trn_rl_repo")

import ml_dtypes  # noqa: E402

from concourse import bacc, bass, mybir, tile  # noqa: E402
from concourse.bass_utils import run_bass_kernel_spmd  # noqa: E402

CORES = 8
K, DK = 4, 16
F = K * DK  # 64 features per node (factors concatenated)
J = 5       # edge slots per row
RPC = 128   # rows per chunk (= partition count)
NPC = 40    # max nodes per chunk
B = 4       # chunks per group (one gather instruction per group)
PT = 5      # projection tiles per macro-tile (NT = CH*NPC/128 is a multiple of 5)
FP32 = mybir.dt.float32
BF16 = mybir.dt.bfloat16
I32 = mybir.dt.int32
BF = ml_dtypes.bfloat16
MAGIC = 0x5F3759DF

LAST_EXEC_NS = None


# ----------------------------------------------------------------- host prep
def _pack(row, col, N):
    """Shard nodes across cores, pack edges into (chunk, row, slot) layout."""
    E = row.shape[0]
    row = np.asarray(row).astype(np.int64)
    col = np.asarray(col).astype(np.int64)

    deg = np.bincount(row, minlength=N)
    cum = np.concatenate([[0], np.cumsum(deg)])
    nb = [0]
    for c in range(1, CORES):
        nb.append(int(np.searchsorted(cum, c * E / CORES)))
    nb.append(N)

    order = np.argsort(row, kind="stable")
    col_s = col[order]

    rcnt = (deg + (J - 1)) // J

    core_chunks = []
    for c in range(CORES):
        nodes = range(nb[c], nb[c + 1])
        chunks, cur, cur_rows = [], [], 0
        for n in nodes:
            r = int(rcnt[n])
            if cur and (cur_rows + r > RPC or len(cur) + 1 > NPC):
                chunks.append(cur)
                cur, cur_rows = [], 0
            cur.append(n)
            cur_rows += r
        if cur:
            chunks.append(cur)
        core_chunks.append(chunks)

    CH = max(len(ch) for ch in core_chunks)
    CH = ((CH + 15) // 16) * 16  # NCpad = CH*NPC divisible by 128; CH % B == 0
    NCpad = CH * NPC
    ZERO = CORES * NCpad  # index of the zero row in the gathered table

    pos_of = np.full(N, -1, dtype=np.int64)
    tau = np.full(CORES * NCpad, -1, dtype=np.int64)
    for c, chunks in enumerate(core_chunks):
        for ci, ch_nodes in enumerate(chunks):
            base = c * NCpad + ci * NPC
            ids = np.asarray(ch_nodes, dtype=np.int64)
            pos_of[ids] = base + np.arange(len(ids))
            tau[base : base + len(ids)] = ids

    tidx = np.full((CORES, CH, RPC, J), ZERO, dtype=np.int32)
    nmap = np.full((CORES, CH, RPC), -1.0, dtype=np.float32)
    for c, chunks in enumerate(core_chunks):
        for ci, ch_nodes in enumerate(chunks):
            r0 = 0
            for pi, n in enumerate(ch_nodes):
                d = int(deg[n])
                if d == 0:
                    continue
                r = int(rcnt[n])
                cols_n = pos_of[col_s[cum[n] : cum[n + 1]]].astype(np.int32)
                buf = np.full(r * J, ZERO, dtype=np.int32)
                buf[:d] = cols_n
                tidx[c, ci, r0 : r0 + r] = buf.reshape(r, J)
                nmap[c, ci, r0 : r0 + r] = float(pi)
                r0 += r

    return dict(CH=CH, NCpad=NCpad, ZERO=ZERO, tau=tau, tidx=tidx, nmap=nmap)


# --------------------------------------------------------------- bass program
def _rsqrt(nc, pool, out, x, shape, name):
    """out = 1/sqrt(x): magic-constant seed + one Newton step (DVE only).

    The bit manipulation runs in fp32 VALUE domain (int32 bits converted to
    float, halved, subtracted from the magic, converted back) because DVE
    arithmetic immediates/scalars are float32-only on HW.
    """
    P, Nf = shape
    iv = pool.tile([P, Nf], FP32, name=f"{name}_iv")
    nc.vector.tensor_copy(iv[:], x.bitcast(I32))
    y0v = pool.tile([P, Nf], FP32, name=f"{name}_y0v")
    nc.vector.tensor_scalar(
        out=y0v[:], in0=iv[:], scalar1=-0.5, scalar2=float(MAGIC),
        op0=mybir.AluOpType.mult, op1=mybir.AluOpType.add)
    y0i = pool.tile([P, Nf], I32, name=f"{name}_y0i")
    nc.vector.tensor_copy(y0i[:], y0v[:])
    y0 = y0i[:].bitcast(FP32)
    a = pool.tile([P, Nf], FP32, name=f"{name}_a")
    nc.vector.tensor_mul(a[:], x, y0)
    nc.vector.tensor_mul(a[:], a[:], y0)
    c = pool.tile([P, Nf], FP32, name=f"{name}_c")
    nc.vector.tensor_scalar(
        out=c[:], in0=a[:], scalar1=-0.5, scalar2=1.5,
        op0=mybir.AluOpType.mult, op1=mybir.AluOpType.add)
    nc.vector.tensor_mul(out, y0, c[:])


def _build(CH, NCpad, n_iter, for_timing=False):
    nc = bacc.Bacc("TRN2", target_bir_lowering=False)
    TBL = CORES * NCpad + 128
    NT = NCpad // RPC
    NMT = NT // PT          # projection macro-tiles
    G = CH // B             # chunk groups
    ACT = mybir.ActivationFunctionType

    embT = nc.dram_tensor("embT", [F, NCpad], FP32, kind="ExternalInput")
    wb_in = nc.dram_tensor("wb", [F, F], FP32, kind="ExternalInput")
    tidx_in = nc.dram_tensor("tidx", [RPC, CH * J], I32, kind="ExternalInput")
    mnr_in = nc.dram_tensor("mnr", [NPC, CH * RPC], BF16, kind="ExternalInput")
    mrn_in = nc.dram_tensor("mrn", [RPC, CH * NPC], BF16, kind="ExternalInput")

    out_t = nc.dram_tensor("out", [NCpad, F], FP32, kind="ExternalOutput")

    fac_loc = nc.dram_tensor("fac_loc", [NCpad, F], BF16)
    fac_full = nc.dram_tensor("fac_full", [TBL, F], BF16,
                              addr_space="Local" if for_timing else "Shared")

    with tile.TileContext(nc) as tc:
        with (
            tc.tile_pool(name="persist", bufs=1) as pp,
            tc.tile_pool(name="sbuf", bufs=4) as pool,
            tc.tile_pool(name="psum", bufs=2, space="PSUM") as psp,
        ):
            # ---- resident tiles
            wb_t = pp.tile([F, F], FP32)
            nc.sync.dma_start(wb_t[:], wb_in[:])
            tidx_t = pp.tile([RPC, CH * J], I32)
            nc.sync.dma_start(tidx_t[:], tidx_in[:])
            zero_t = pp.tile([RPC, F], BF16)
            nc.vector.memset(zero_t[:], 0.0)
            nc.sync.dma_start(fac_full[CORES * NCpad :, :], zero_t[:])

            # ---- P0: projection of the core's node slice (512-node macro-tiles)
            for t in range(NMT):
                emb_t = pool.tile([F, PT * RPC], FP32, name="emb_t")
                nc.sync.dma_start(
                    emb_t[:], embT[:, t * PT * RPC : (t + 1) * PT * RPC])
                proj_ps = psp.tile([RPC, PT, F], FP32, name="proj_ps", space="PSUM")
                for s in range(PT):
                    nc.tensor.matmul(
                        out=proj_ps[:, s, :],
                        lhsT=emb_t[:, s * RPC : (s + 1) * RPC], rhs=wb_t[:],
                        start=True, stop=True)
                lr2 = pool.tile([RPC, PT, F], FP32, name="lr2")
                nc.vector.tensor_scalar_mul(lr2[:], proj_ps[:], 0.2)
                lr = pool.tile([RPC, PT, F], FP32, name="lr")
                nc.vector.tensor_max(lr[:], proj_ps[:], lr2[:])
                sq = pool.tile([RPC, PT, F], FP32, name="sq")
                nc.scalar.activation(sq[:], lr[:], ACT.Square)
                ss = pool.tile([RPC, PT * K], FP32, name="ss")
                nc.vector.tensor_reduce(
                    out=ss[:], in_=sq[:].rearrange("p s (k d) -> p (s k) d", k=K),
                    axis=mybir.AxisListType.X, op=mybir.AluOpType.add)
                rn = pool.tile([RPC, PT * K], FP32, name="rn")
                _rsqrt(nc, pool, rn[:], ss[:], (RPC, PT * K), "prn")
                fac_t = pool.tile([RPC, PT, F], BF16, name="fac_t")
                nc.vector.tensor_mul(
                    fac_t[:].rearrange("p s (k d) -> p s k d", k=K),
                    lr[:].rearrange("p s (k d) -> p s k d", k=K),
                    rn[:].rearrange("p (s k) -> p s k", k=K)
                        .unsqueeze(3).broadcast_to([RPC, PT, K, DK]))
                nc.sync.dma_start(
                    fac_loc[t * PT * RPC : (t + 1) * PT * RPC, :]
                        .rearrange("(s p) f -> p s f", s=PT),
                    fac_t[:])

            # ---- AllGather fac -> fac_full[0 : CORES*NCpad]
            if for_timing:
                nc.sync.dma_start(fac_full[0:NCpad, :], fac_loc[:])
            else:
                nc.gpsimd.collective_compute(
                    "AllGather", mybir.AluOpType.bypass,
                    replica_groups=[list(range(CORES))],
                    ins=[fac_loc[:]], outs=[fac_full[0 : CORES * NCpad, :]],
                )

            # ---- routing iterations, fused per group of B chunks
            BJ = B * J
            for g in range(G):
                tails = pool.tile([RPC, B, J, F], BF16, name="tails")
                for m in range(BJ):
                    nc.gpsimd.indirect_dma_start(
                        out=tails[:].rearrange("p b j f -> p (b j) f")[:, m, :],
                        out_offset=None,
                        in_=fac_full[:],
                        in_offset=bass.IndirectOffsetOnAxis(
                            ap=tidx_t[:, g * BJ + m : g * BJ + m + 1], axis=0),
                    )
                facw = pool.tile([NPC, B, F], BF16, name="facw")
                nc.sync.dma_start(
                    facw[:],
                    fac_loc[g * B * NPC : (g + 1) * B * NPC, :]
                        .rearrange("(b n) f -> n b f", b=B))
                mnr_t = pool.tile([NPC, B, RPC], BF16, name="mnr_t")
                nc.sync.dma_start(
                    mnr_t[:], mnr_in[:, g * B * RPC : (g + 1) * B * RPC])
                mrn_t = pool.tile([RPC, B, NPC], BF16, name="mrn_t")
                nc.sync.dma_start(
                    mrn_t[:], mrn_in[:, g * B * NPC : (g + 1) * B * NPC])
                nf_bf = pool.tile([NPC, B, F], BF16, name="nf_bf")

                for it in range(n_iter):
                    win = facw if it == 0 else nf_bf
                    head_ps = psp.tile([RPC, B, F], FP32, name="head_ps",
                                       space="PSUM")
                    for b in range(B):
                        nc.tensor.matmul(
                            out=head_ps[:, b, :], lhsT=mnr_t[:, b, :],
                            rhs=win[:, b, :], start=True, stop=True)
                    head_bf = pool.tile([RPC, B, F], BF16, name="head_bf")
                    nc.scalar.activation(head_bf[:], head_ps[:], ACT.Copy)

                    prod = pool.tile([RPC, B, J, F], BF16, name="prod")
                    nc.vector.tensor_mul(
                        prod[:], tails[:],
                        head_bf[:].unsqueeze(2).broadcast_to([RPC, B, J, F]))
                    pko = pool.tile([RPC, BJ, K], FP32, name="pko")
                    nc.vector.tensor_reduce(
                        out=pko[:],
                        in_=prod[:].rearrange("p b j (k d) -> p (b j) k d", k=K),
                        axis=mybir.AxisListType.X, op=mybir.AluOpType.add)
                    expp = pool.tile([RPC, BJ, K], FP32, name="expp")
                    nc.scalar.activation(expp[:], pko[:], ACT.Exp)
                    den = pool.tile([RPC, BJ], FP32, name="den")
                    nc.vector.tensor_reduce(
                        out=den[:], in_=expp[:], axis=mybir.AxisListType.X,
                        op=mybir.AluOpType.add)
                    rden = pool.tile([RPC, BJ], FP32, name="rden")
                    nc.vector.reciprocal(rden[:], den[:])
                    pn = pool.tile([RPC, BJ, K], BF16, name="pn")
                    nc.vector.tensor_mul(
                        pn[:], expp[:],
                        rden[:].unsqueeze(2).broadcast_to([RPC, BJ, K]))
                    msg = pool.tile([RPC, B, J, F], BF16, name="msg")
                    nc.vector.tensor_mul(
                        msg[:].rearrange("p b j (k d) -> p (b j) k d", k=K),
                        tails[:].rearrange("p b j (k d) -> p (b j) k d", k=K),
                        pn[:].unsqueeze(3).broadcast_to([RPC, BJ, K, DK]))
                    s01 = pool.tile([RPC, B, F], BF16, name="s01")
                    nc.vector.tensor_add(s01[:], msg[:, :, 0, :], msg[:, :, 1, :])
                    s23 = pool.tile([RPC, B, F], BF16, name="s23")
                    nc.vector.tensor_add(s23[:], msg[:, :, 2, :], msg[:, :, 3, :])
                    nc.vector.tensor_add(s01[:], s01[:], s23[:])
                    part = pool.tile([RPC, B, F], BF16, name="part")
                    nc.vector.tensor_add(part[:], s01[:], msg[:, :, 4, :])

                    agg_ps = psp.tile([NPC, B, F], FP32, name="agg_ps",
                                      space="PSUM")
                    for b in range(B):
                        nc.tensor.matmul(
                            out=agg_ps[:, b, :], lhsT=mrn_t[:, b, :],
                            rhs=part[:, b, :], start=True, stop=True)
                    upd = pool.tile([NPC, B, F], FP32, name="upd")
                    nc.vector.tensor_add(upd[:], agg_ps[:], facw[:])
                    usq = pool.tile([NPC, B, F], FP32, name="usq")
                    nc.scalar.activation(usq[:], upd[:], ACT.Square)
                    uss = pool.tile([NPC, B * K], FP32, name="uss")
                    nc.vector.tensor_reduce(
                        out=uss[:],
                        in_=usq[:].rearrange("n b (k d) -> n (b k) d", k=K),
                        axis=mybir.AxisListType.X, op=mybir.AluOpType.add)
                    urn = pool.tile([NPC, B * K], FP32, name="urn")
                    _rsqrt(nc, pool, urn[:], uss[:], (NPC, B * K), "urn")
                    urn_b = (urn[:].rearrange("n (b k) -> n b k", k=K)
                             .unsqueeze(3).broadcast_to([NPC, B, K, DK]))
                    if it < n_iter - 1:
                        nc.vector.tensor_mul(
                            nf_bf[:].rearrange("n b (k d) -> n b k d", k=K),
                            upd[:].rearrange("n b (k d) -> n b k d", k=K),
                            urn_b)
                    else:
                        nf_out = pool.tile([NPC, B, F], FP32, name="nf_out")
                        nc.vector.tensor_mul(
                            nf_out[:].rearrange("n b (k d) -> n b k d", k=K),
                            upd[:].rearrange("n b (k d) -> n b k d", k=K),
                            urn_b)
                        nc.sync.dma_start(
                            out_t[g * B * NPC : (g + 1) * B * NPC, :]
                                .rearrange("(b n) f -> n b f", b=B),
                            nf_out[:])

    nc.compile()
    return nc


_CACHE = {}


def kernel(all_emb, W, b, row, col, iter_k):
    all_emb = np.asarray(all_emb, dtype=np.float32)
    W = np.asarray(W, dtype=np.float32)
    b = np.asarray(b, dtype=np.float32)
    row = np.asarray(row)
    col = np.asarray(col)
    n_iter = int(iter_k)
    N = all_emb.shape[0]

    pk = _pack(row, col, N)
    CH, NCpad, tau = pk["CH"], pk["NCpad"], pk["tau"]

    key = (N, CH, NCpad, n_iter)
    if key not in _CACHE:
        _CACHE[key] = _build(CH, NCpad, n_iter)
    nc = _CACHE[key]

    wb_np = np.ascontiguousarray(
        np.transpose(W + b, (1, 0, 2)).reshape(W.shape[1], F)).astype(np.float32)

    iota = np.arange(NPC, dtype=np.float32)
    in_maps = []
    for c in range(CORES):
        sl = tau[c * NCpad : (c + 1) * NCpad]
        embT_c = np.zeros((F, NCpad), dtype=np.float32)
        valid = sl >= 0
        embT_c[:, valid] = all_emb[sl[valid]].T
        tidx_c = pk["tidx"][c]          # [CH, RPC, J]
        tidxT = np.ascontiguousarray(tidx_c.transpose(1, 0, 2).reshape(RPC, CH * J))
        nmap = pk["nmap"][c]            # [CH, RPC]
        onehot = (nmap[None, :, :] == iota[:, None, None])  # [NPC, CH, RPC]
        mnr = np.ascontiguousarray(onehot.reshape(NPC, CH * RPC)).astype(BF)
        mrn = np.ascontiguousarray(
            onehot.transpose(2, 1, 0).reshape(RPC, CH * NPC)).astype(BF)
        in_maps.append({
            "embT": embT_c, "wb": wb_np, "tidx": tidxT, "mnr": mnr, "mrn": mrn,
        })

    bkr = run_bass_kernel_spmd(
        nc, in_maps, list(range(CORES)),
        tmpdir=os.environ.get("BASS_TRACE_DIR") or None,
    )
    global LAST_EXEC_NS
    LAST_EXEC_NS = bkr.exec_time_ns
    res = bkr.results
    full = np.concatenate([res[c]["out"] for c in range(CORES)], axis=0)
    out = np.empty((N, F), dtype=np.float32)
    valid = tau >= 0
    out[tau[valid]] = full[valid]
    return out


# revision 27
# speedup vs baseline: 1.1600x; 1.1600x over previous
"""DisenGCN Trainium2 kernel (8 NeuronCores, Bass/Tile), v2.

Strategy
--------
Nodes are sharded across 8 cores by contiguous ranges balanced on in-edge
count.  Host-side index prep packs each node's in-edges into fixed-width
rows (J slots) and packs nodes into 128-row chunks (<= NPC nodes per
chunk); chunks are processed in groups of B=4.  On device:

  - projection fac = l2norm(leaky_relu(emb @ (W+b))) runs per 512-node
    macro-tile; the per-factor rsqrt uses a DVE integer bit-trick plus one
    Newton step (keeps the ACT engine pinned to the `exp_and_others`
    activation-table set: Prelu/Square/Exp/Copy only -- no table reloads).
  - fac is AllGathered once in bf16; per group, B*J single-offset indirect
    DMAs fetch the tail rows (one [128,1]-offset gather per slot column --
    multi-offset indirect DMA mis-executes on real HW).
  - both routing iterations run back-to-back per group: tails and the
    fac window live in SBUF, iteration 0's normalized output feeds
    iteration 1's head matmul directly -- no DRAM round-trip.
  - heads are expanded from the <=NPC-node window with a one-hot PE
    matmul; per-node aggregation uses the transposed one-hot.  Both
    one-hot matrices are precomputed on the host in bf16 and streamed.
  - all big elementwise/reduce work runs in bf16 on DVE (2x packing);
    softmax exp and the squares run on ACT.

The only collective is one AllGather of the projected bf16 fac table.
"""

import os
import sys

import numpy as np

sys.path.insert(0, "/opt/trn_rl_repo")

import ml_dtypes  # noqa: E402

from concourse import bacc, bass, mybir, tile  # noqa: E402
from concourse.bass_utils import run_bass_kernel_spmd  # noqa: E402

CORES = 8
K, DK = 4, 16
F = K * DK  # 64 features per node (factors concatenated)
J = 5       # edge slots per row
RPC = 128   # rows per chunk (= partition count)
NPC = 40    # max nodes per chunk
B = 4       # chunks per group (one gather instruction per group)
PT = 5      # projection tiles per macro-tile (NT = CH*NPC/128 is a multiple of 5)
FP32 = mybir.dt.float32
BF16 = mybir.dt.bfloat16
I32 = mybir.dt.int32
BF = ml_dtypes.bfloat16
MAGIC = 0x5F3759DF

LAST_EXEC_NS = None


# ----------------------------------------------------------------- host prep
def _pack(row, col, N):
    """Shard nodes across cores, pack edges into (chunk, row, slot) layout."""
    E = row.shape[0]
    row = np.asarray(row).astype(np.int64)
    col = np.asarray(col).astype(np.int64)

    deg = np.bincount(row, minlength=N)
    cum = np.concatenate([[0], np.cumsum(deg)])
    nb = [0]
    for c in range(1, CORES):
        nb.append(int(np.searchsorted(cum, c * E / CORES)))
    nb.append(N)

    order = np.argsort(row, kind="stable")
    col_s = col[order]

    rcnt = (deg + (J - 1)) // J

    core_chunks = []
    for c in range(CORES):
        nodes = range(nb[c], nb[c + 1])
        chunks, cur, cur_rows = [], [], 0
        for n in nodes:
            r = int(rcnt[n])
            if cur and (cur_rows + r > RPC or len(cur) + 1 > NPC):
                chunks.append(cur)
                cur, cur_rows = [], 0
            cur.append(n)
            cur_rows += r
        if cur:
            chunks.append(cur)
        core_chunks.append(chunks)

    CH = max(len(ch) for ch in core_chunks)
    CH = ((CH + 15) // 16) * 16  # NCpad = CH*NPC divisible by 128; CH % B == 0
    NCpad = CH * NPC
    ZERO = CORES * NCpad  # index of the zero row in the gathered table

    pos_of = np.full(N, -1, dtype=np.int64)
    tau = np.full(CORES * NCpad, -1, dtype=np.int64)
    for c, chunks in enumerate(core_chunks):
        for ci, ch_nodes in enumerate(chunks):
            base = c * NCpad + ci * NPC
            ids = np.asarray(ch_nodes, dtype=np.int64)
            pos_of[ids] = base + np.arange(len(ids))
            tau[base : base + len(ids)] = ids

    tidx = np.full((CORES, CH, RPC, J), ZERO, dtype=np.int32)
    nmap = np.full((CORES, CH, RPC), -1.0, dtype=np.float32)
    for c, chunks in enumerate(core_chunks):
        for ci, ch_nodes in enumerate(chunks):
            r0 = 0
            for pi, n in enumerate(ch_nodes):
                d = int(deg[n])
                if d == 0:
                    continue
                r = int(rcnt[n])
                cols_n = pos_of[col_s[cum[n] : cum[n + 1]]].astype(np.int32)
                buf = np.full(r * J, ZERO, dtype=np.int32)
                buf[:d] = cols_n
                tidx[c, ci, r0 : r0 + r] = buf.reshape(r, J)
                nmap[c, ci, r0 : r0 + r] = float(pi)
                r0 += r

    return dict(CH=CH, NCpad=NCpad, ZERO=ZERO, tau=tau, tidx=tidx, nmap=nmap)


# --------------------------------------------------------------- bass program
def _rsqrt(nc, pool, out, x, shape, name):
    """out = 1/sqrt(x): magic-constant seed + one Newton step (DVE only).

    The bit manipulation runs in fp32 VALUE domain (int32 bits converted to
    float, halved, subtracted from the magic, converted back) because DVE
    arithmetic immediates/scalars are float32-only on HW.
    """
    P, Nf = shape
    iv = pool.tile([P, Nf], FP32, name=f"{name}_iv")
    nc.vector.tensor_copy(iv[:], x.bitcast(I32))
    y0v = pool.tile([P, Nf], FP32, name=f"{name}_y0v")
    nc.vector.tensor_scalar(
        out=y0v[:], in0=iv[:], scalar1=-0.5, scalar2=float(MAGIC),
        op0=mybir.AluOpType.mult, op1=mybir.AluOpType.add)
    y0i = pool.tile([P, Nf], I32, name=f"{name}_y0i")
    nc.vector.tensor_copy(y0i[:], y0v[:])
    y0 = y0i[:].bitcast(FP32)
    a = pool.tile([P, Nf], FP32, name=f"{name}_a")
    nc.vector.tensor_mul(a[:], x, y0)
    nc.vector.tensor_mul(a[:], a[:], y0)
    c = pool.tile([P, Nf], FP32, name=f"{name}_c")
    nc.vector.tensor_scalar(
        out=c[:], in0=a[:], scalar1=-0.5, scalar2=1.5,
        op0=mybir.AluOpType.mult, op1=mybir.AluOpType.add)
    nc.vector.tensor_mul(out, y0, c[:])


def _build(CH, NCpad, n_iter, for_timing=False):
    nc = bacc.Bacc("TRN2", target_bir_lowering=False)
    TBL = CORES * NCpad + 128
    NT = NCpad // RPC
    NMT = NT // PT          # projection macro-tiles
    G = CH // B             # chunk groups
    ACT = mybir.ActivationFunctionType

    embT = nc.dram_tensor("embT", [F, NCpad], FP32, kind="ExternalInput")
    wb_in = nc.dram_tensor("wb", [F, F], FP32, kind="ExternalInput")
    tidx_in = nc.dram_tensor("tidx", [RPC, CH * J], I32, kind="ExternalInput")
    mnr_in = nc.dram_tensor("mnr", [NPC, CH * RPC], BF16, kind="ExternalInput")
    mrn_in = nc.dram_tensor("mrn", [RPC, CH * NPC], BF16, kind="ExternalInput")

    out_t = nc.dram_tensor("out", [NCpad, F], FP32, kind="ExternalOutput")

    fac_loc = nc.dram_tensor("fac_loc", [NCpad, F], BF16)
    fac_full = nc.dram_tensor("fac_full", [TBL, F], BF16,
                              addr_space="Local" if for_timing else "Shared")

    with tile.TileContext(nc) as tc:
        with (
            tc.tile_pool(name="persist", bufs=1) as pp,
            tc.tile_pool(name="sbuf", bufs=3) as pool,
            tc.tile_pool(name="tailsp", bufs=6) as tlp,
            tc.tile_pool(name="psum", bufs=2, space="PSUM") as psp,
        ):
            # ---- resident tiles
            wb_t = pp.tile([F, F], FP32)
            nc.sync.dma_start(wb_t[:], wb_in[:])
            tidx_t = pp.tile([RPC, CH * J], I32)
            nc.sync.dma_start(tidx_t[:], tidx_in[:])
            zero_t = pp.tile([RPC, F], BF16)
            nc.vector.memset(zero_t[:], 0.0)
            nc.sync.dma_start(fac_full[CORES * NCpad :, :], zero_t[:])

            # ---- P0: projection of the core's node slice (512-node macro-tiles)
            for t in range(NMT):
                emb_t = pool.tile([F, PT * RPC], FP32, name="emb_t")
                nc.sync.dma_start(
                    emb_t[:], embT[:, t * PT * RPC : (t + 1) * PT * RPC])
                proj_ps = psp.tile([RPC, PT, F], FP32, name="proj_ps", space="PSUM")
                for s in range(PT):
                    nc.tensor.matmul(
                        out=proj_ps[:, s, :],
                        lhsT=emb_t[:, s * RPC : (s + 1) * RPC], rhs=wb_t[:],
                        start=True, stop=True)
                lr2 = pool.tile([RPC, PT, F], FP32, name="lr2")
                nc.vector.tensor_scalar_mul(lr2[:], proj_ps[:], 0.2)
                lr = pool.tile([RPC, PT, F], FP32, name="lr")
                nc.vector.tensor_max(lr[:], proj_ps[:], lr2[:])
                sq = pool.tile([RPC, PT, F], FP32, name="sq")
                nc.scalar.activation(sq[:], lr[:], ACT.Square)
                ss = pool.tile([RPC, PT * K], FP32, name="ss")
                nc.vector.tensor_reduce(
                    out=ss[:], in_=sq[:].rearrange("p s (k d) -> p (s k) d", k=K),
                    axis=mybir.AxisListType.X, op=mybir.AluOpType.add)
                rn = pool.tile([RPC, PT * K], FP32, name="rn")
                _rsqrt(nc, pool, rn[:], ss[:], (RPC, PT * K), "prn")
                fac_t = pool.tile([RPC, PT, F], BF16, name="fac_t")
                nc.vector.tensor_mul(
                    fac_t[:].rearrange("p s (k d) -> p s k d", k=K),
                    lr[:].rearrange("p s (k d) -> p s k d", k=K),
                    rn[:].rearrange("p (s k) -> p s k", k=K)
                        .unsqueeze(3).broadcast_to([RPC, PT, K, DK]))
                nc.sync.dma_start(
                    fac_loc[t * PT * RPC : (t + 1) * PT * RPC, :]
                        .rearrange("(s p) f -> p s f", s=PT),
                    fac_t[:])

            # ---- AllGather fac -> fac_full[0 : CORES*NCpad]
            if for_timing:
                nc.sync.dma_start(fac_full[0:NCpad, :], fac_loc[:])
            else:
                nc.gpsimd.collective_compute(
                    "AllGather", mybir.AluOpType.bypass,
                    replica_groups=[list(range(CORES))],
                    ins=[fac_loc[:]], outs=[fac_full[0 : CORES * NCpad, :]],
                )

            # ---- routing iterations, fused per group of B chunks
            BJ = B * J
            for g in range(G):
                tails = tlp.tile([RPC, B, J, F], BF16, name="tails")
                for m in range(BJ):
                    nc.gpsimd.indirect_dma_start(
                        out=tails[:].rearrange("p b j f -> p (b j) f")[:, m, :],
                        out_offset=None,
                        in_=fac_full[:],
                        in_offset=bass.IndirectOffsetOnAxis(
                            ap=tidx_t[:, g * BJ + m : g * BJ + m + 1], axis=0),
                    )
                facw = pool.tile([NPC, B, F], BF16, name="facw")
                nc.sync.dma_start(
                    facw[:],
                    fac_loc[g * B * NPC : (g + 1) * B * NPC, :]
                        .rearrange("(b n) f -> n b f", b=B))
                mnr_t = pool.tile([NPC, B, RPC], BF16, name="mnr_t")
                nc.sync.dma_start(
                    mnr_t[:], mnr_in[:, g * B * RPC : (g + 1) * B * RPC])
                mrn_t = pool.tile([RPC, B, NPC], BF16, name="mrn_t")
                nc.sync.dma_start(
                    mrn_t[:], mrn_in[:, g * B * NPC : (g + 1) * B * NPC])
                nf_bf = pool.tile([NPC, B, F], BF16, name="nf_bf")

                for it in range(n_iter):
                    win = facw if it == 0 else nf_bf
                    head_ps = psp.tile([RPC, B, F], FP32, name="head_ps",
                                       space="PSUM")
                    for b in range(B):
                        nc.tensor.matmul(
                            out=head_ps[:, b, :], lhsT=mnr_t[:, b, :],
                            rhs=win[:, b, :], start=True, stop=True)
                    head_bf = pool.tile([RPC, B, F], BF16, name="head_bf")
                    nc.scalar.activation(head_bf[:], head_ps[:], ACT.Copy)

                    prod = pool.tile([RPC, B, J, F], BF16, name="prod")
                    nc.vector.tensor_mul(
                        prod[:], tails[:],
                        head_bf[:].unsqueeze(2).broadcast_to([RPC, B, J, F]))
                    # bf16 out keeps the reduce in the 2x packed DVE mode;
                    # the DVE accumulates internally in fp32.
                    pko = pool.tile([RPC, BJ, K], BF16, name="pko")
                    with nc.allow_low_precision("bf16 dot output, fp32 accum"):
                        nc.vector.tensor_reduce(
                            out=pko[:],
                            in_=prod[:].rearrange("p b j (k d) -> p (b j) k d",
                                                  k=K),
                            axis=mybir.AxisListType.X, op=mybir.AluOpType.add)
                    expp = pool.tile([RPC, BJ, K], FP32, name="expp")
                    nc.scalar.activation(expp[:], pko[:], ACT.Exp)
                    den = pool.tile([RPC, BJ], FP32, name="den")
                    nc.vector.tensor_reduce(
                        out=den[:], in_=expp[:], axis=mybir.AxisListType.X,
                        op=mybir.AluOpType.add)
                    rden = pool.tile([RPC, BJ], FP32, name="rden")
                    nc.vector.reciprocal(rden[:], den[:])
                    pn = pool.tile([RPC, BJ, K], BF16, name="pn")
                    nc.vector.tensor_mul(
                        pn[:], expp[:],
                        rden[:].unsqueeze(2).broadcast_to([RPC, BJ, K]))
                    msg = pool.tile([RPC, B, J, F], BF16, name="msg")
                    nc.vector.tensor_mul(
                        msg[:].rearrange("p b j (k d) -> p (b j) k d", k=K),
                        tails[:].rearrange("p b j (k d) -> p (b j) k d", k=K),
                        pn[:].unsqueeze(3).broadcast_to([RPC, BJ, K, DK]))
                    s01 = pool.tile([RPC, B, F], BF16, name="s01")
                    nc.vector.tensor_add(s01[:], msg[:, :, 0, :], msg[:, :, 1, :])
                    s23 = pool.tile([RPC, B, F], BF16, name="s23")
                    nc.vector.tensor_add(s23[:], msg[:, :, 2, :], msg[:, :, 3, :])
                    nc.vector.tensor_add(s01[:], s01[:], s23[:])
                    part = pool.tile([RPC, B, F], BF16, name="part")
                    nc.vector.tensor_add(part[:], s01[:], msg[:, :, 4, :])

                    agg_ps = psp.tile([NPC, B, F], FP32, name="agg_ps",
                                      space="PSUM")
                    for b in range(B):
                        nc.tensor.matmul(
                            out=agg_ps[:, b, :], lhsT=mrn_t[:, b, :],
                            rhs=part[:, b, :], start=True, stop=True)
                    upd = pool.tile([NPC, B, F], FP32, name="upd")
                    nc.vector.tensor_add(upd[:], agg_ps[:], facw[:])
                    usq = pool.tile([NPC, B, F], FP32, name="usq")
                    nc.scalar.activation(usq[:], upd[:], ACT.Square)
                    uss = pool.tile([NPC, B * K], FP32, name="uss")
                    nc.vector.tensor_reduce(
                        out=uss[:],
                        in_=usq[:].rearrange("n b (k d) -> n (b k) d", k=K),
                        axis=mybir.AxisListType.X, op=mybir.AluOpType.add)
                    urn = pool.tile([NPC, B * K], FP32, name="urn")
                    _rsqrt(nc, pool, urn[:], uss[:], (NPC, B * K), "urn")
                    urn_b = (urn[:].rearrange("n (b k) -> n b k", k=K)
                             .unsqueeze(3).broadcast_to([NPC, B, K, DK]))
                    if it < n_iter - 1:
                        nc.vector.tensor_mul(
                            nf_bf[:].rearrange("n b (k d) -> n b k d", k=K),
                            upd[:].rearrange("n b (k d) -> n b k d", k=K),
                            urn_b)
                    else:
                        nf_out = pool.tile([NPC, B, F], FP32, name="nf_out")
                        nc.vector.tensor_mul(
                            nf_out[:].rearrange("n b (k d) -> n b k d", k=K),
                            upd[:].rearrange("n b (k d) -> n b k d", k=K),
                            urn_b)
                        nc.sync.dma_start(
                            out_t[g * B * NPC : (g + 1) * B * NPC, :]
                                .rearrange("(b n) f -> n b f", b=B),
                            nf_out[:])

    nc.compile()
    return nc


_CACHE = {}


def kernel(all_emb, W, b, row, col, iter_k):
    all_emb = np.asarray(all_emb, dtype=np.float32)
    W = np.asarray(W, dtype=np.float32)
    b = np.asarray(b, dtype=np.float32)
    row = np.asarray(row)
    col = np.asarray(col)
    n_iter = int(iter_k)
    N = all_emb.shape[0]

    pk = _pack(row, col, N)
    CH, NCpad, tau = pk["CH"], pk["NCpad"], pk["tau"]

    key = (N, CH, NCpad, n_iter)
    if key not in _CACHE:
        _CACHE[key] = _build(CH, NCpad, n_iter)
    nc = _CACHE[key]

    wb_np = np.ascontiguousarray(
        np.transpose(W + b, (1, 0, 2)).reshape(W.shape[1], F)).astype(np.float32)

    iota = np.arange(NPC, dtype=np.float32)
    in_maps = []
    for c in range(CORES):
        sl = tau[c * NCpad : (c + 1) * NCpad]
        embT_c = np.zeros((F, NCpad), dtype=np.float32)
        valid = sl >= 0
        embT_c[:, valid] = all_emb[sl[valid]].T
        tidx_c = pk["tidx"][c]          # [CH, RPC, J]
        tidxT = np.ascontiguousarray(tidx_c.transpose(1, 0, 2).reshape(RPC, CH * J))
        nmap = pk["nmap"][c]            # [CH, RPC]
        onehot = (nmap[None, :, :] == iota[:, None, None])  # [NPC, CH, RPC]
        mnr = np.ascontiguousarray(onehot.reshape(NPC, CH * RPC)).astype(BF)
        mrn = np.ascontiguousarray(
            onehot.transpose(2, 1, 0).reshape(RPC, CH * NPC)).astype(BF)
        in_maps.append({
            "embT": embT_c, "wb": wb_np, "tidx": tidxT, "mnr": mnr, "mrn": mrn,
        })

    bkr = run_bass_kernel_spmd(
        nc, in_maps, list(range(CORES)),
        tmpdir=os.environ.get("BASS_TRACE_DIR") or None,
    )
    global LAST_EXEC_NS
    LAST_EXEC_NS = bkr.exec_time_ns
    res = bkr.results
    full = np.concatenate([res[c]["out"] for c in range(CORES)], axis=0)
    out = np.empty((N, F), dtype=np.float32)
    valid = tau >= 0
    out[tau[valid]] = full[valid]
    return out
